# revision 1
# baseline (speedup 1.0000x reference)
"""Trainium2 Bass kernel for nn_ATTMILLoss.

Reference computation:
    rows[b,n,:]  = syb_graph[b, idx_of_objs[b,n], :]            (gather)
    pos[k,b,n]   = sum_l att[k,b,n,l] * (rows[b,n,l] > 0)
    neg[k,b,n]   = sum_l att[k,b,n,l] * (rows[b,n,l] == 0)
    loss         = mean(relu(MARGIN - (pos - neg)))

Since rows in {0,1}: pos - neg = sum_l att[k,b,n,l] * (2*rows[b,n,l] - 1).

Strategy (8 cores, data-parallel over batch):
  Each core gets 16 batches. Per batch: indirect-DMA gather of the 512
  selected syb_graph rows (laid out n = 4*p + ni so att slab loads are
  8KiB-contiguous per partition, issued two batches ahead), then one
  fused affine_mul_reduce per (block, n-chunk) on the vector engine:
  diff = sum_l (2*rows - 1) * att, which folds the int->(+-1) conversion
  into the multiply+reduce. att slabs stream in 2 MiB DMAs alternating
  the two HWDGE rings (SP/ACT). Final relu+sum on the scalar engine;
  per-core [128] partials are summed on the host.

  Measured ~360 us on HW; DMA-bound at ~112 MiB/core streaming
  (16 SDMA engines ~87% busy), vector engine ~77% busy.
"""

import sys

for _p in ("/opt/trn_rl_repo",):
    if _p not in sys.path:
        sys.path.insert(0, _p)

import numpy as np

BLOCKS, BATCH, N, L = 6, 128, 512, 512
MARGIN = 0.6
NCORES = 8
BPC = BATCH // NCORES  # batches per core
P = 128
NCHUNK = N // P  # 4 n-chunks per slab; n = NCHUNK*p + ni
KGRP = 2  # att slabs per DMA

_CACHE = {}


def _build_program():
    import concourse.bacc as bacc
    import concourse.bass as bass
    import concourse.mybir as mybir
    import concourse.tile as tile

    nc = bacc.Bacc("TRN2", target_bir_lowering=False, debug=False)

    att = nc.dram_tensor(
        "att", [BLOCKS, BPC, N, L], mybir.dt.float32, kind="ExternalInput"
    )
    syb = nc.dram_tensor("syb", [BPC, N, L], mybir.dt.int32, kind="ExternalInput")
    idx = nc.dram_tensor(
        "idx", [P, BPC, NCHUNK], mybir.dt.int32, kind="ExternalInput"
    )
    out = nc.dram_tensor("out", [P, 1], mybir.dt.float32, kind="ExternalOutput")

    ncols = BPC * BLOCKS * NCHUNK  # one diff column per (b, k, ni)

    with tile.TileContext(nc) as tc:
        with (
            tc.tile_pool(name="idxp", bufs=1) as idxp,
            tc.tile_pool(name="rowsp", bufs=4) as rowsp,
            tc.tile_pool(name="attp", bufs=5) as attp,
            tc.tile_pool(name="scrp", bufs=2) as scrp,
            tc.tile_pool(name="accp", bufs=1) as accp,
        ):
            margin_t = idxp.tile([P, 1], mybir.dt.float32)
            nc.gpsimd.memset(margin_t[:], MARGIN)

            # All gather indices up front, pre-packed on host to
            # idx[p, b, f] = idx_of_objs[b, 4p+f] so the load is one
            # contiguous DMA.
            idx_t = idxp.tile([P, BPC, NCHUNK], mybir.dt.int32)
            nc.sync.dma_start(out=idx_t[:], in_=idx[:])

            D = accp.tile([P, ncols], mybir.dt.float32)

            # Gather the selected syb_graph rows for each batch.
            # rows[p, ni*L : (ni+1)*L] = syb[b, idx[b, 4p+ni], :]
            # One offset per destination partition per indirect DMA
            # (HW semantics), so one gather per n-chunk. Emitted
            # PIPELINE_AHEAD batches ahead of their consumers so the
            # rows are resident before the vector engine needs them.
            PIPELINE_AHEAD = 2
            rows_tiles = {}

            def emit_gather(b):
                rows = rowsp.tile([P, NCHUNK * L], mybir.dt.int32, tag="rows")
                rows_tiles[b] = rows
                for ni in range(NCHUNK):
                    nc.gpsimd.indirect_dma_start(
                        out=rows[:, ni * L : (ni + 1) * L],
                        out_offset=None,
                        in_=syb[:],
                        in_offset=bass.IndirectOffsetOnAxis(
                            ap=idx_t[:, b, ni : ni + 1], axis=1
                        ),
                        element_offset=b * N * L,
                    )

            for b in range(min(PIPELINE_AHEAD + 1, BPC)):
                emit_gather(b)

            for b in range(BPC):
                rows = rows_tiles.pop(b)
                if b + PIPELINE_AHEAD + 1 < BPC:
                    emit_gather(b + PIPELINE_AHEAD + 1)
                for k0 in range(0, BLOCKS, KGRP):
                    # KGRP att slabs per DMA: [KGRP, 512, 512] ->
                    # [128, KGRP, 2048]; partition p gets rows 4p..4p+3 of
                    # each slab, 8KiB contiguous per run. Alternate the two
                    # HWDGE rings (SP / ACT) for issue.
                    att_t = attp.tile([P, KGRP, NCHUNK * L], mybir.dt.float32)
                    eng = nc.sync if (b * (BLOCKS // KGRP) + k0 // KGRP) % 2 == 0 else nc.scalar
                    eng.dma_start(
                        out=att_t[:],
                        in_=att[k0 : k0 + KGRP, b].rearrange(
                            "k (p f) l -> p k (f l)", p=P
                        ),
                    )
                    for kk in range(KGRP):
                        for ni in range(NCHUNK):
                            scr = scrp.tile([P, L], mybir.dt.float32)
                            col = (b * BLOCKS + k0 + kk) * NCHUNK + ni
                            # diff = sum_l (2*rows - 1) * att in one DVE op
                            nc.vector.affine_mul_reduce(
                                out=scr[:],
                                accum_out=D[:, col : col + 1],
                                in0=rows[:, ni * L : (ni + 1) * L],
                                in1=att_t[:, kk, ni * L : (ni + 1) * L],
                                scale=2.0,
                                bias=-1.0,
                            )

            # partial[p] = sum_cols relu(MARGIN - D)
            relu_t = accp.tile([P, ncols], mybir.dt.float32)
            partial = accp.tile([P, 1], mybir.dt.float32)
            nc.scalar.activation(
                out=relu_t[:],
                in_=D[:],
                func=mybir.ActivationFunctionType.Relu,
                scale=-1.0,
                bias=margin_t[:],
                accum_out=partial[:],
            )
            nc.sync.dma_start(out=out[:], in_=partial[:])

    nc.compile()
    return nc


def _get_program():
    if "nc" not in _CACHE:
        _CACHE["nc"] = _build_program()
    return _CACHE["nc"]


def _pack_idx(idx_shard):
    # [BPC, N] -> [P, BPC, NCHUNK] with idx[p, b, f] = idx_shard[b, 4p+f]
    return np.ascontiguousarray(
        idx_shard.reshape(BPC, P, NCHUNK).transpose(1, 0, 2)
    )


def _shard_inputs(idx_of_objs, syb_graph, att_weights):
    in_maps = []
    for c in range(NCORES):
        sl = slice(c * BPC, (c + 1) * BPC)
        in_maps.append(
            {
                "att": np.ascontiguousarray(att_weights[:, sl]),
                "syb": np.ascontiguousarray(syb_graph[sl]),
                "idx": _pack_idx(idx_of_objs[sl]),
            }
        )
    return in_maps


def kernel(idx_of_objs, valid2all, syb_graph, att_weights, vis_len):
    from concourse.bass_utils import run_bass_kernel_spmd

    del valid2all, vis_len  # no-ops given the reference's setup
    idx_of_objs = np.asarray(idx_of_objs, dtype=np.int32)
    syb_graph = np.asarray(syb_graph, dtype=np.int32)
    att_weights = np.asarray(att_weights, dtype=np.float32)

    nc = _get_program()
    in_maps = _shard_inputs(idx_of_objs, syb_graph, att_weights)
    res = run_bass_kernel_spmd(nc, in_maps, list(range(NCORES)))
    total = 0.0
    for r in res.results:
        total += float(np.asarray(r["out"], dtype=np.float64).sum())
    loss = total / (BLOCKS * BATCH * N)
    return np.float32(loss)

